# revision 16
# baseline (speedup 1.0000x reference)
import sys
import numpy as np

sys.path.insert(0, "/opt/trn_rl_repo")

import concourse.bass as bass  # noqa: E402
import concourse.mybir as mybir  # noqa: E402
import concourse.tile as tile  # noqa: E402
from concourse import bacc  # noqa: E402
from concourse.bass_utils import run_bass_kernel_spmd  # noqa: E402
from contextlib import ExitStack  # noqa: E402

F32 = mybir.dt.float32
F32R = mybir.dt.float32r
ACTF = mybir.ActivationFunctionType
ALU = mybir.AluOpType

NCORES = 8
B, D, H1, H2, KH = 16384, 512, 1024, 2048, 11
BC = B // NCORES            # 2048 rows per core
EPS = 1e-5
THR = 0.25
LN10 = float(np.log(10.0))
N_INV = 1.0 / float(B)

_BUILD_CACHE = {}
TRACE = False          # test harness hook: set True to capture a profile
LAST_RESULT = None     # test harness hook: BassKernelResults of last run


def _build():
    if "nc" in _BUILD_CACHE:
        return _BUILD_CACHE["nc"]
    nc = bacc.Bacc(None, target_bir_lowering=False, debug=False)

    # ------- external I/O (per core) -------
    xn_d = nc.dram_tensor("xn", [128, 16, 512], F32, kind="ExternalInput")
    xt_d = nc.dram_tensor("xt", [128, 4, 2048], F32, kind="ExternalInput")
    wc1_d = nc.dram_tensor("wc1", [128, 4, H1], F32, kind="ExternalInput")
    wc2_d = nc.dram_tensor("wc2", [128, 8, H2], F32, kind="ExternalInput")
    wc3_d = nc.dram_tensor("wc3", [128, 16, KH], F32, kind="ExternalInput")
    rw1_d = nc.dram_tensor("rw1", [KH, 128, 4, H1], F32, kind="ExternalInput")
    rw2_d = nc.dram_tensor("rw2", [KH, 128, 8, 512], F32, kind="ExternalInput")
    rw3_d = nc.dram_tensor("rw3", [128, 4, KH], F32, kind="ExternalInput")
    vecl1_d = nc.dram_tensor("vecl1", [128, 96, 2], F32, kind="ExternalInput")
    vecl2_d = nc.dram_tensor("vecl2", [128, 60, 2], F32, kind="ExternalInput")
    row11_d = nc.dram_tensor("row11", [128, 4, KH], F32, kind="ExternalInput")

    of_d = nc.dram_tensor("of", [128, 16], F32, kind="ExternalOutput")
    ol_d = nc.dram_tensor("ol", [128, 16, KH], F32, kind="ExternalOutput")
    op_d = nc.dram_tensor("op", [128, 16, KH], F32, kind="ExternalOutput")
    oa_d = nc.dram_tensor("oa", [128, 16], F32, kind="ExternalOutput")

    # ------- internal DRAM -------
    gin = nc.dram_tensor("gin", [128, 2052], F32, kind="Internal")
    gout = nc.dram_tensor("gout", [128, 2052], F32, kind="Internal", addr_space="Shared")
    sin = nc.dram_tensor("sin", [128, 120], F32, kind="Internal")
    sout = nc.dram_tensor("sout", [128, 120], F32, kind="Internal", addr_space="Shared")
    h2s = nc.dram_tensor("h2s", [128, 16, 2048], F32, kind="Internal")
    r2s = nc.dram_tensor("r2s", [KH, 128, 4, 2048], F32, kind="Internal")

    RG = [list(range(NCORES))]

    with tile.TileContext(nc) as tc, ExitStack() as top:
        const = top.enter_context(tc.tile_pool(name="const", bufs=1))
        ones = const.tile([128, 1], F32)
        nc.vector.memset(ones, 1.0)
        epsT = const.tile([128, 1], F32)
        nc.vector.memset(epsT, EPS)
        row11_sb = const.tile([128, 4, KH], F32)
        nc.sync.dma_start(row11_sb[:], row11_d[:])
        vecl1_sb = const.tile([128, 96, 2], F32)
        nc.sync.dma_start(vecl1_sb[:], vecl1_d[:])
        vecl2_sb = const.tile([128, 60, 2], F32)
        nc.sync.dma_start(vecl2_sb[:], vecl2_d[:])
        stats_sb = const.tile([128, 60, 2], F32)
        # per-matrix L1 bn scale/shift
        a1c = const.tile([128, 8], F32)
        b1c = const.tile([128, 8], F32)
        A1H = const.tile([128, KH, 8], F32)
        B1H = const.tile([128, KH, 8], F32)
        # L2 bn scale/shift (after AR2)
        a2c = const.tile([128, 16], F32)
        b2c = const.tile([128, 16], F32)
        A2H = const.tile([128, KH, 4], F32)
        B2H = const.tile([128, KH, 4], F32)
        p_sb = const.tile([128, 16, KH], F32)
        pcts_sb = const.tile([128, 16, KH], F32)
        logits_sb = const.tile([128, 16, KH], F32)
        final_sb = const.tile([128, 16], F32)
        atpm_sb = const.tile([128, 16], F32)

        # ================= phase A: Gram + colsum =================
        # pool scopes are strict LIFO: a1T outlives wc1/xt32, which outlive G
        sA1 = top.enter_context(ExitStack())      # a1T: closed after C-L2
        a1pool = sA1.enter_context(tc.tile_pool(name="a1pool", bufs=1))
        a1T = a1pool.tile([128, 8, 2048], F32)

        sW = top.enter_context(ExitStack())       # wc1+xt32: closed after C-L1
        pcW = sW.enter_context(tc.tile_pool(name="pcW", bufs=1))
        wc1_sb = pcW.tile([128, 4, H1], F32)
        nc.sync.dma_start(wc1_sb[:], wc1_d[:])
        xt32_sb = pcW.tile([128, 4, 2048], F32)
        nc.sync.dma_start(xt32_sb[:], xt_d[:])

        sG = sW.enter_context(ExitStack())        # G: closed after B
        gpool = sG.enter_context(tc.tile_pool(name="gpool", bufs=1))
        G32g = gpool.tile([128, 4, 512], F32)
        Gr = gpool.tile([128, 4, 512], F32R)
        csg = gpool.tile([128, 4], F32)

        with ExitStack() as sA:
            pa = sA.enter_context(tc.tile_pool(name="pa", bufs=1))
            psA = sA.enter_context(tc.tile_pool(name="psA", bufs=1, space="PSUM"))
            xn_sb = pa.tile([128, 16, 512], F32)
            nc.sync.dma_start(xn_sb[:], xn_d[:])
            G32 = pa.tile([128, 4, 512], F32)
            cs0 = pa.tile([128, 4], F32)
            for m in range(4):
                psg = psA.tile([128, 512], F32, tag=f"g{m}")
                for t in range(16):
                    nc.tensor.matmul(psg[:], xn_sb[:, t, m * 128:(m + 1) * 128],
                                     xn_sb[:, t, :], start=(t == 0), stop=(t == 15))
                nc.vector.tensor_copy(G32[:, m, :], psg[:])
            for m in range(4):
                psc = psA.tile([128, 1], F32, tag=f"c{m}")
                for t in range(16):
                    nc.tensor.matmul(psc[:], xn_sb[:, t, m * 128:(m + 1) * 128],
                                     ones[:], start=(t == 0), stop=(t == 15))
                nc.vector.tensor_copy(cs0[:, m:m + 1], psc[:])
            nc.gpsimd.dma_start(gin[:, 0:2048], G32[:].rearrange("p m d -> p (m d)"))
            nc.gpsimd.dma_start(gin[:, 2048:2052], cs0[:])

        nc.gpsimd.collective_compute(
            "AllReduce", ALU.add, replica_groups=RG,
            ins=[gin[:].opt()], outs=[gout[:].opt()])
        nc.sync.dma_start(G32g[:].rearrange("p m d -> p (m d)"), gout[:, 0:2048])
        nc.sync.dma_start(Gr[:].rearrange("p m d -> p (m d)"),
                          gout[:, 0:2048].bitcast(F32R))
        nc.sync.dma_start(csg[:], gout[:, 2048:2052])

        # ================= phase B: L1 stats for all 12 matrices =================
        # q_j = sum_d1 W[d1,j] * (G @ W)[d1,j]  (sum of h^2 over full batch)
        # s_j = (W^T cs)_j                      (sum of h over full batch)
        def emit_l1_stats(W32, Wr, n_jt, voff, alphaT, betaT, use_f32r, pools):
            tmpP, stP, psP = pools
            Hn = n_jt * 128
            tmp = tmpP.tile([128, 4, Hn], F32, tag="tmp")
            qs = stP.tile([128, n_jt, 2], F32, tag="qs")
            for m4 in range(4):
                for nch in range(Hn // 512):
                    psgw = psP.tile([128, 512], F32, tag="gw")
                    for kt in range(4):
                        nc.tensor.matmul(
                            psgw[:],
                            (Gr if use_f32r else G32g)[:, kt, m4 * 128:(m4 + 1) * 128],
                            (Wr if use_f32r else W32)[:, kt, nch * 512:(nch + 1) * 512],
                            start=(kt == 0), stop=(kt == 3))
                    nc.vector.tensor_tensor(
                        tmp[:, m4, nch * 512:(nch + 1) * 512], psgw[:],
                        W32[:, m4, nch * 512:(nch + 1) * 512], ALU.mult)
            for jc in range(n_jt):
                psq = psP.tile([128, 1], F32, tag="q")
                for m4 in range(4):
                    nc.tensor.matmul(psq[:], tmp[:, m4, jc * 128:(jc + 1) * 128],
                                     ones[:], start=(m4 == 0), stop=(m4 == 3))
                pss = psP.tile([128, 1], F32, tag="s")
                for kt in range(4):
                    nc.tensor.matmul(pss[:], W32[:, kt, jc * 128:(jc + 1) * 128],
                                     csg[:, kt:kt + 1], start=(kt == 0), stop=(kt == 3))
                nc.vector.tensor_copy(qs[:, jc, 0:1], pss[:])
                nc.vector.tensor_copy(qs[:, jc, 1:2], psq[:])
            mean = stP.tile([128, n_jt], F32, tag="mean")
            var = stP.tile([128, n_jt], F32, tag="var")
            m2t = stP.tile([128, n_jt], F32, tag="m2t")
            nc.vector.tensor_scalar_mul(mean[:], qs[:, :, 0], N_INV)
            nc.vector.tensor_scalar_mul(var[:], qs[:, :, 1], N_INV)
            nc.vector.tensor_tensor(m2t[:], mean[:], mean[:], ALU.mult)
            nc.vector.tensor_sub(var[:], var[:], m2t[:])
            nc.scalar.activation(var[:], var[:], ACTF.Sqrt, bias=epsT[:])
            nc.vector.reciprocal(var[:], var[:])
            nc.vector.tensor_tensor(alphaT, vecl1_sb[:, voff:voff + n_jt, 0],
                                    var[:], ALU.mult)
            nc.vector.tensor_tensor(m2t[:], mean[:], alphaT, ALU.mult)
            nc.vector.tensor_sub(betaT, vecl1_sb[:, voff:voff + n_jt, 1], m2t[:])

        with ExitStack() as sB:
            tmpP = sB.enter_context(tc.tile_pool(name="tmpP", bufs=1))
            stP = sB.enter_context(tc.tile_pool(name="stP", bufs=2))
            psB = sB.enter_context(tc.tile_pool(name="psB", bufs=1, space="PSUM"))
            wkB = sB.enter_context(tc.tile_pool(name="wkB", bufs=2))
            pools = (tmpP, stP, psB)
            emit_l1_stats(wc1_sb, None, 8, 0, a1c[:], b1c[:], False, pools)
            for k in range(KH):
                wkr = wkB.tile([128, 4, H1], F32R, tag="wkr")
                nc.sync.dma_start(wkr[:], rw1_d[k].bitcast(F32R))
                emit_l1_stats(wkr[:].bitcast(F32), wkr, 8, 8 + 8 * k,
                              A1H[:, k, :], B1H[:, k, :], True, pools)
        sG.close()      # G no longer needed; free its SBUF

        # ================= phase C: classifier =================
        # L1 matmuls, raw eviction (bn applied in-place once stats land)
        with ExitStack() as sC1:
            psC1 = sC1.enter_context(tc.tile_pool(name="psC1", bufs=4, space="PSUM"))
            for jc in range(8):
                for nb in range(4):
                    ps = psC1.tile([128, 512], F32, tag="c")
                    for kt in range(4):
                        nc.tensor.matmul(ps[:], wc1_sb[:, kt, jc * 128:(jc + 1) * 128],
                                         xt32_sb[:, kt, nb * 512:(nb + 1) * 512],
                                         start=(kt == 0), stop=(kt == 3))
                    nc.vector.tensor_copy(a1T[:, jc, nb * 512:(nb + 1) * 512], ps[:])
            for jc in range(8):
                nc.scalar.activation(a1T[:, jc, :], a1T[:, jc, :], ACTF.Relu,
                                     scale=a1c[:, jc:jc + 1], bias=b1c[:, jc:jc + 1])
        sW.close()      # wc1 + xt32 no longer needed

        # L2: h2pre -> dram scratch + per-core stats
        with ExitStack() as sC:
            psC = sC.enter_context(tc.tile_pool(name="psC", bufs=4, space="PSUM"))
            wstream = sC.enter_context(tc.tile_pool(name="wstream", bufs=3))
            hchunk = sC.enter_context(tc.tile_pool(name="hchunk", bufs=4))
            stC = sC.enter_context(tc.tile_pool(name="stC", bufs=4))
            for jc2 in range(16):
                mv6 = stC.tile([128, 4, 6], F32, tag="mv6")
                wts = []
                for kt in range(8):
                    wt = wstream.tile([128, 128], F32, tag=f"wc2_{kt}")
                    nc.sync.dma_start(wt[:], wc2_d[:, kt, jc2 * 128:(jc2 + 1) * 128])
                    wts.append(wt)
                for nb in range(4):
                    ps = psC.tile([128, 512], F32, tag="c")
                    for kt in range(8):
                        nc.tensor.matmul(ps[:], wts[kt][:],
                                         a1T[:, kt, nb * 512:(nb + 1) * 512],
                                         start=(kt == 0), stop=(kt == 7))
                    hc = hchunk.tile([128, 512], F32, tag="hc")
                    nc.vector.tensor_copy(hc[:], ps[:])
                    nc.vector.bn_stats(mv6[:, nb, :], hc[:])
                    nc.sync.dma_start(h2s[:, jc2, nb * 512:(nb + 1) * 512], hc[:])
                mv = stC.tile([128, 2], F32, tag="mv")
                nc.vector.bn_aggr(mv[:], mv6[:])
                tq = stC.tile([128, 1], F32, tag="tq")
                nc.vector.tensor_tensor(tq[:], mv[:, 0:1], mv[:, 0:1], ALU.mult)
                nc.vector.tensor_add(tq[:], tq[:], mv[:, 1:2])
                nc.vector.tensor_scalar_mul(stats_sb[:, jc2, 1:2], tq[:], float(BC))
                nc.vector.tensor_scalar_mul(stats_sb[:, jc2, 0:1], mv[:, 0:1], float(BC))
        sA1.close()     # a1T no longer needed

        # ================= phase D: 11 reg heads (f32r) =================
        with ExitStack() as sD:
            pxr = sD.enter_context(tc.tile_pool(name="pxr", bufs=1))
            wkD = sD.enter_context(tc.tile_pool(name="wkD", bufs=2))
            a1kP = sD.enter_context(tc.tile_pool(name="a1kP", bufs=1))
            psD = sD.enter_context(tc.tile_pool(name="psD", bufs=4, space="PSUM"))
            rchunk = sD.enter_context(tc.tile_pool(name="rchunk", bufs=4))
            stD = sD.enter_context(tc.tile_pool(name="stD", bufs=4))

            xtr_sb = pxr.tile([128, 4, 2048], F32R)
            nc.sync.dma_start(xtr_sb[:], xt_d[:].bitcast(F32R))

            for k in range(KH):
                w1r = wkD.tile([128, 4, H1], F32R, tag="w1r")
                nc.sync.dma_start(w1r[:], rw1_d[k].bitcast(F32R))
                w2r = wkD.tile([128, 8, 512], F32R, tag="w2r")
                nc.sync.dma_start(w2r[:], rw2_d[k].bitcast(F32R))
                a1k = a1kP.tile([128, 8, 2048], F32R, tag="a1k")
                for jc in range(8):
                    for nb in range(4):
                        ps = psD.tile([128, 512], F32, tag="d")
                        for kt in range(4):
                            nc.tensor.matmul(ps[:], w1r[:, kt, jc * 128:(jc + 1) * 128],
                                             xtr_sb[:, kt, nb * 512:(nb + 1) * 512],
                                             start=(kt == 0), stop=(kt == 3))
                        nc.scalar.activation(a1k[:, jc, nb * 512:(nb + 1) * 512],
                                             ps[:], ACTF.Relu,
                                             scale=A1H[:, k, jc:jc + 1],
                                             bias=B1H[:, k, jc:jc + 1])
                for jc2 in range(4):
                    mv6 = stD.tile([128, 4, 6], F32, tag="mv6")
                    for nb in range(4):
                        ps = psD.tile([128, 512], F32, tag="d")
                        for kt in range(8):
                            nc.tensor.matmul(ps[:], w2r[:, kt, jc2 * 128:(jc2 + 1) * 128],
                                             a1k[:, kt, nb * 512:(nb + 1) * 512],
                                             start=(kt == 0), stop=(kt == 7))
                        rc = rchunk.tile([128, 512], F32, tag="rc")
                        nc.vector.tensor_copy(rc[:], ps[:])
                        nc.vector.bn_stats(mv6[:, nb, :], rc[:])
                        nc.sync.dma_start(r2s[k][:, jc2, nb * 512:(nb + 1) * 512], rc[:])
                    mv = stD.tile([128, 2], F32, tag="mv")
                    nc.vector.bn_aggr(mv[:], mv6[:])
                    tq = stD.tile([128, 1], F32, tag="tq")
                    nc.vector.tensor_tensor(tq[:], mv[:, 0:1], mv[:, 0:1], ALU.mult)
                    nc.vector.tensor_add(tq[:], tq[:], mv[:, 1:2])
                    col = 16 + 4 * k + jc2
                    nc.vector.tensor_scalar_mul(stats_sb[:, col, 1:2], tq[:], float(BC))
                    nc.vector.tensor_scalar_mul(stats_sb[:, col, 0:1], mv[:, 0:1],
                                                float(BC))

        # ================= AR2: reduce L2 stats =================
        nc.gpsimd.dma_start(sin[:], stats_sb[:].rearrange("p c t -> p (c t)"))
        nc.gpsimd.collective_compute(
            "AllReduce", ALU.add, replica_groups=RG,
            ins=[sin[:].opt()], outs=[sout[:].opt()])

        with ExitStack() as sAB:
            stE = sAB.enter_context(tc.tile_pool(name="stE", bufs=1))
            statg = stE.tile([128, 60, 2], F32)
            nc.sync.dma_start(statg[:].rearrange("p c t -> p (c t)"), sout[:])
            meanA = stE.tile([128, 60], F32)
            varA = stE.tile([128, 60], F32)
            m2A = stE.tile([128, 60], F32)
            nc.vector.tensor_scalar_mul(meanA[:], statg[:, :, 0], N_INV)
            nc.vector.tensor_scalar_mul(varA[:], statg[:, :, 1], N_INV)
            nc.vector.tensor_tensor(m2A[:], meanA[:], meanA[:], ALU.mult)
            nc.vector.tensor_sub(varA[:], varA[:], m2A[:])
            nc.scalar.activation(varA[:], varA[:], ACTF.Sqrt, bias=epsT[:])
            nc.vector.reciprocal(varA[:], varA[:])
            alphaA = stE.tile([128, 60], F32)
            betaA = stE.tile([128, 60], F32)
            nc.vector.tensor_tensor(alphaA[:], vecl2_sb[:, :, 0], varA[:], ALU.mult)
            nc.vector.tensor_tensor(m2A[:], meanA[:], alphaA[:], ALU.mult)
            nc.vector.tensor_sub(betaA[:], vecl2_sb[:, :, 1], m2A[:])
            nc.vector.tensor_copy(a2c[:], alphaA[:, 0:16])
            nc.vector.tensor_copy(b2c[:], betaA[:, 0:16])
            for k in range(KH):
                nc.vector.tensor_copy(A2H[:, k, :], alphaA[:, 16 + 4 * k:20 + 4 * k])
                nc.vector.tensor_copy(B2H[:, k, :], betaA[:, 16 + 4 * k:20 + 4 * k])

        # ================= phase E-cls: a2, logits, softmax =================
        with ExitStack() as sE:
            pE = sE.enter_context(tc.tile_pool(name="pE", bufs=3))
            psE = sE.enter_context(tc.tile_pool(name="psE", bufs=1, space="PSUM"))
            smE = sE.enter_context(tc.tile_pool(name="smE", bufs=4))
            wc3_sb = pE.tile([128, 16, KH], F32, tag="wc3")
            nc.sync.dma_start(wc3_sb[:], wc3_d[:])
            for wave in range(2):
                psl = [psE.tile([128, KH], F32, tag=f"lg{i}", name=f"psl{i}")
                       for i in range(8)]
                for jc2 in range(16):
                    a2row = pE.tile([128, 2048], F32, tag="a2row")
                    nc.sync.dma_start(a2row[:], h2s[:, jc2, :])
                    nc.scalar.activation(a2row[:], a2row[:], ACTF.Relu,
                                         scale=a2c[:, jc2:jc2 + 1],
                                         bias=b2c[:, jc2:jc2 + 1])
                    for i in range(8):
                        bc = wave * 8 + i
                        nc.tensor.matmul(psl[i][:],
                                         a2row[:, bc * 128:(bc + 1) * 128],
                                         wc3_sb[:, jc2, :],
                                         start=(jc2 == 0), stop=(jc2 == 15))
                for i in range(8):
                    bc = wave * 8 + i
                    nc.vector.tensor_tensor(logits_sb[:, bc, :], psl[i][:],
                                            row11_sb[:, 0, :], ALU.add)
                    mx = smE.tile([128, 1], F32, tag="mx")
                    nc.vector.tensor_reduce(mx[:], logits_sb[:, bc, :],
                                            mybir.AxisListType.X, ALU.max)
                    et = smE.tile([128, KH], F32, tag="et")
                    nc.vector.tensor_scalar(et[:], logits_sb[:, bc, :], mx[:], None,
                                            ALU.subtract)
                    nc.scalar.activation(et[:], et[:], ACTF.Exp)
                    sm = smE.tile([128, 1], F32, tag="sm")
                    nc.vector.tensor_reduce(sm[:], et[:], mybir.AxisListType.X, ALU.add)
                    nc.vector.reciprocal(sm[:], sm[:])
                    nc.vector.tensor_scalar_mul(p_sb[:, bc, :], et[:], sm[:])
            nc.sync.dma_start(ol_d[:], logits_sb[:])

        # ================= phase E-heads: a2k, r3, sigmoid =================
        with ExitStack() as sF:
            pF = sF.enter_context(tc.tile_pool(name="pF", bufs=2))
            psF = sF.enter_context(tc.tile_pool(name="psF", bufs=4, space="PSUM"))
            rw3_sb = pF.tile([128, 4, KH], F32, tag="rw3")
            nc.sync.dma_start(rw3_sb[:], rw3_d[:])
            for k in range(KH):
                a2k = pF.tile([128, 4, 2048], F32, tag="a2k")
                nc.sync.dma_start(a2k[:], r2s[k][:])
                for jc2 in range(4):
                    nc.scalar.activation(a2k[:, jc2, :], a2k[:, jc2, :], ACTF.Relu,
                                         scale=A2H[:, k, jc2:jc2 + 1],
                                         bias=B2H[:, k, jc2:jc2 + 1])
                for bc in range(16):
                    ps = psF.tile([128, 1], F32, tag="r3")
                    for jc2 in range(4):
                        nc.tensor.matmul(ps[:],
                                         a2k[:, jc2, bc * 128:(bc + 1) * 128],
                                         rw3_sb[:, jc2, k:k + 1],
                                         start=(jc2 == 0), stop=(jc2 == 3))
                    nc.scalar.activation(pcts_sb[:, bc, k:k + 1], ps[:], ACTF.Sigmoid,
                                         bias=row11_sb[:, 1, k:k + 1])
            nc.sync.dma_start(op_d[:], pcts_sb[:])

        # ================= phase F: combine =================
        with ExitStack() as sG:
            pG = sG.enter_context(tc.tile_pool(name="pG", bufs=4))
            for bc in range(16):
                p = p_sb[:, bc, :]
                q = pcts_sb[:, bc, :]
                valid = pG.tile([128, KH], F32, tag="valid")
                nc.vector.tensor_scalar(valid[:], p, THR, None, ALU.is_gt)
                hv = pG.tile([128, 1], F32, tag="hv")
                nc.vector.tensor_reduce(hv[:], valid[:], mybir.AxisListType.X, ALU.max)
                m8 = pG.tile([128, 8], F32, tag="m8")
                nc.vector.max(m8[:], p)
                top3 = pG.tile([128, KH], F32, tag="top3")
                nc.vector.tensor_scalar(top3[:], p, m8[:, 2:3], None, ALU.is_ge)
                # eff = top3 + (valid - top3) * hv
                eff = pG.tile([128, KH], F32, tag="eff")
                nc.vector.tensor_sub(eff[:], valid[:], top3[:])
                nc.vector.tensor_scalar_mul(eff[:], eff[:], hv[:])
                nc.vector.tensor_add(eff[:], eff[:], top3[:])
                w = pG.tile([128, KH], F32, tag="w")
                nc.vector.tensor_tensor(w[:], p, eff[:], ALU.mult)
                ws = pG.tile([128, 1], F32, tag="ws")
                nc.vector.tensor_reduce(ws[:], w[:], mybir.AxisListType.X, ALU.add)
                nc.vector.reciprocal(ws[:], ws[:])
                atp = pG.tile([128, KH], F32, tag="atp")
                nc.vector.tensor_tensor(atp[:], q, row11_sb[:, 2, :], ALU.add)
                nc.scalar.activation(atp[:], atp[:], ACTF.Exp, scale=LN10)
                nc.vector.tensor_tensor(w[:], w[:], atp[:], ALU.mult)
                num = pG.tile([128, 1], F32, tag="num")
                nc.vector.tensor_reduce(num[:], w[:], mybir.AxisListType.X, ALU.add)
                nc.vector.tensor_tensor(final_sb[:, bc:bc + 1], num[:], ws[:], ALU.mult)
                # argmax bin -> atp_max
                sel = pG.tile([128, KH], F32, tag="sel")
                nc.vector.tensor_scalar(sel[:], p, m8[:, 0:1], None, ALU.is_ge)
                pm = pG.tile([128, 1], F32, tag="pm")
                t2 = pG.tile([128, KH], F32, tag="t2")
                nc.vector.tensor_tensor(t2[:], q, sel[:], ALU.mult)
                nc.vector.tensor_reduce(pm[:], t2[:], mybir.AxisListType.X, ALU.add)
                lm = pG.tile([128, 1], F32, tag="lm")
                nc.vector.tensor_tensor(t2[:], row11_sb[:, 2, :], sel[:], ALU.mult)
                nc.vector.tensor_reduce(lm[:], t2[:], mybir.AxisListType.X, ALU.add)
                nc.vector.tensor_add(pm[:], pm[:], lm[:])
                nc.scalar.activation(atpm_sb[:, bc:bc + 1], pm[:], ACTF.Exp, scale=LN10)
            nc.sync.dma_start(of_d[:], final_sb[:])
            nc.sync.dma_start(oa_d[:], atpm_sb[:])

    nc.compile()
    _BUILD_CACHE["nc"] = nc
    return nc


def _pack_kt(W, kt):
    # [kt*128, M] -> [128, kt, M]
    W = np.ascontiguousarray(np.asarray(W, np.float32))
    return np.ascontiguousarray(W.reshape(kt, 128, W.shape[1]).transpose(1, 0, 2))


def _pack_vec(v, t):
    # [t*128] -> [128, t]
    return np.ascontiguousarray(np.asarray(v, np.float32).reshape(t, 128).T)


def kernel(x, Wc1, bc1, gc1, bec1, Wc2, bc2, gc2, bec2, Wc3, bc3,
           Rw1, Rb1, Rg1, Rbe1, Rw2, Rb2, Rg2, Rbe2, Rw3, Rb3):
    nc = _build()
    f = np.float32
    x = np.asarray(x, f)

    wc1 = _pack_kt(Wc1, 4)
    wc2 = _pack_kt(Wc2, 8)
    wc3 = _pack_kt(Wc3, 16)
    rw1 = np.ascontiguousarray(np.stack([_pack_kt(np.asarray(Rw1)[k], 4)
                                         for k in range(KH)]))
    rw2 = np.ascontiguousarray(np.stack([_pack_kt(np.asarray(Rw2)[k], 8)
                                         for k in range(KH)]))
    rw3 = _pack_kt(np.asarray(Rw3, f)[:, :, 0].T, 4)     # [512, 11] -> [128,4,11]

    vecl1 = np.zeros((128, 96, 2), f)
    vecl1[:, 0:8, 0] = _pack_vec(gc1, 8)
    vecl1[:, 0:8, 1] = _pack_vec(bec1, 8)
    for k in range(KH):
        vecl1[:, 8 + 8 * k:16 + 8 * k, 0] = _pack_vec(np.asarray(Rg1)[k], 8)
        vecl1[:, 8 + 8 * k:16 + 8 * k, 1] = _pack_vec(np.asarray(Rbe1)[k], 8)
    vecl2 = np.zeros((128, 60, 2), f)
    vecl2[:, 0:16, 0] = _pack_vec(gc2, 16)
    vecl2[:, 0:16, 1] = _pack_vec(bec2, 16)
    for k in range(KH):
        vecl2[:, 16 + 4 * k:20 + 4 * k, 0] = _pack_vec(np.asarray(Rg2)[k], 4)
        vecl2[:, 16 + 4 * k:20 + 4 * k, 1] = _pack_vec(np.asarray(Rbe2)[k], 4)
    row11 = np.zeros((128, 4, KH), f)
    row11[:, 0, :] = np.asarray(bc3, f)
    row11[:, 1, :] = np.asarray(Rb3, f)[:, 0]
    row11[:, 2, :] = np.arange(KH, dtype=f) - 6.0

    common = dict(wc1=wc1, wc2=wc2, wc3=wc3, rw1=rw1, rw2=rw2, rw3=rw3,
                  vecl1=vecl1, vecl2=vecl2, row11=row11)
    in_maps = []
    for c in range(NCORES):
        xc = x[c * BC:(c + 1) * BC]                      # [2048, 512]
        xn = np.ascontiguousarray(xc.reshape(16, 128, 512).transpose(1, 0, 2))
        xt = np.ascontiguousarray(xc.T.reshape(4, 128, 2048).transpose(1, 0, 2))
        in_maps.append(dict(common, xn=xn, xt=xt))

    res = run_bass_kernel_spmd(nc, in_maps, core_ids=list(range(NCORES)),
                               trace=TRACE)
    global LAST_RESULT
    LAST_RESULT = res

    finals, logits, pcts, atpms = [], [], [], []
    for c in range(NCORES):
        r = res.results[c]
        finals.append(r["of"].T.reshape(BC, 1))
        logits.append(r["ol"].transpose(1, 0, 2).reshape(BC, KH))
        pcts.append(r["op"].transpose(1, 0, 2).reshape(BC, KH))
        atpms.append(r["oa"].T.reshape(BC, 1))
    return (np.ascontiguousarray(np.concatenate(finals)),
            np.ascontiguousarray(np.concatenate(logits)),
            np.ascontiguousarray(np.concatenate(pcts)),
            np.ascontiguousarray(np.concatenate(atpms)))
